# revision 19
# baseline (speedup 1.0000x reference)
"""Trainium2 Bass kernel for nn_Attention_63660005261999.

Reference (per batch element b):
    c = concat(mems[:, b, :], h[:, b, :])           # [klen, d]
    S = h_b @ c_b.T                                  # [qlen, klen]
    S[q, k] = -1e6  where k > q + mlen               # causal w/ memory
    P = softmax(S, axis=-1)
    out_b = P @ c_b                                  # [qlen, d]

Sharding: bsz=8 across 8 NeuronCores, one batch element per core.

Per-core design (two-phase flash attention, fp32 I/O, float32r matmuls):
  build: stream c (mems+h) natural tiles once, PE-transpose into cT
         (d-on-partition) stored in internal DRAM; retain first
         CMEM_RESIDENT k-tiles of natural-layout c in SBUF (f32r view).
  per q-superblock of 4 q-blocks (128 q each):
    QK:  S[qb, :klen_r] accumulated in PSUM over 8 d-chunks,
         lhsT = cT slice (queries), rhs = cT k-tile (keys); running
         per-tile max on DVE, S tiles copied to SBUF by ACT.
         Causal boundary handled by a gpsimd affine_select on the last
         512-wide k-tile; k-tiles beyond the boundary are skipped.
    exp: ACT activation Exp with bias = -rowmax, accum_out = rowsum.
    PV:  per pair of q-blocks: for each 128-wide k-chunk, PE-transpose
         P block, round to f32r in the PSUM->SBUF copy, matmul against
         natural-layout c tile; accumulate O in PSUM; final DVE
         tensor_scalar multiply by 1/rowsum on the way out.

The walrus build in this container accepts at most ONE sync-wait per
instruction; split_waits() rewrites the scheduled module so extra waits
ride on dedicated same-engine NoOps.
"""

import numpy as np
from contextlib import ExitStack

import concourse.bass as bass
import concourse.mybir as mybir
import concourse.tile as tile
from concourse.bass_utils import run_bass_kernel_spmd
from concourse.masks import make_identity

F32 = mybir.dt.float32
F32R = mybir.dt.float32r
NEG_INF = -1000000.0

QLEN, MLEN, BSZ, D = 2048, 2048, 8, 1024
N_CORES = 8
CMEM_RESIDENT = 12  # natural-layout c k-tiles kept resident in SBUF


def split_waits(nc, max_waits: int = 1) -> int:
    """walrus here allows at most one sync wait per instruction; move extras
    onto preceding same-engine NoOp carriers."""
    n_split = 0
    for f in nc.m.functions:
        for blk in f.blocks:
            new_instrs = []
            for ins in blk.instructions:
                si = getattr(ins, "sync_info", None)
                if si is not None and si.on_wait and len(si.on_wait) > max_waits:
                    waits = list(si.on_wait)
                    keep = waits[-max_waits:]
                    spill = waits[:-max_waits]
                    for j, w in enumerate(spill):
                        nop = mybir.InstNoOp(
                            name=f"{ins.name}_wf{j}",
                            text_hint="waitfix",
                            bass_nofuse=True,
                        )
                        nop.engine = ins.engine
                        nop.sync_info = mybir.SyncInfo(on_wait=[w], on_update=[])
                        nc.register_instruction(nop, overwrite=True)
                        new_instrs.append(nop)
                    ins.sync_info = mybir.SyncInfo(
                        on_wait=keep, on_update=list(si.on_update)
                    )
                    n_split += 1
                new_instrs.append(ins)
            blk.instructions[:] = new_instrs
    return n_split


def build_attention(qlen=QLEN, mlen=MLEN, d=D, cmem_resident=CMEM_RESIDENT,
                    q_super=4):
    """One-core attention program: inputs h [qlen, d], mems [mlen, d]."""
    klen = qlen + mlen
    DC = d // 128            # d-chunks
    QB = qlen // 128         # q-blocks
    KB = klen // 128         # k-chunks (natural layout)
    KM = mlen // 128         # k-chunks in mems
    NKT = klen // 512        # 512-wide k-tiles (max)
    assert qlen % 512 == 0 and mlen % 512 == 0 and d % 128 == 0

    def klen_valid(i):       # number of unmasked keys for q-block i
        return mlen + 128 * (i + 1)

    def klen_r(i):           # rounded up to 512-wide k-tiles
        return ((klen_valid(i) + 511) // 512) * 512

    nc = bass.Bass()
    h_dram = nc.declare_dram_parameter("h", [qlen, d], F32, isOutput=False)
    m_dram = nc.declare_dram_parameter("mems", [mlen, d], F32, isOutput=False)
    o_dram = nc.declare_dram_parameter("out", [qlen, d], F32, isOutput=True)
    # one scratch tensor per 512-wide key group so Tile's DRAM dependency
    # tracking (whole-tensor granularity) doesn't serialize QK behind the
    # entire build phase
    ct_g = [nc.dram_tensor(f"ct_g{g}", [DC, 128, 512], F32R)
            for g in range(klen // 512)]

    with tile.TileContext(nc) as tc, ExitStack() as ctx:
        p_cn = ctx.enter_context(tc.tile_pool(name="cn", bufs=3))
        p_cmem = ctx.enter_context(tc.tile_pool(name="cmem", bufs=max(cmem_resident, 1)))
        p_ctkt = ctx.enter_context(tc.tile_pool(name="ctkt", bufs=2 * (d // 128)))
        p_ht = ctx.enter_context(tc.tile_pool(name="ht", bufs=d // 128))
        p_srow = ctx.enter_context(tc.tile_pool(name="srow", bufs=q_super))
        p_pt = ctx.enter_context(tc.tile_pool(name="pt", bufs=4))
        p_ost = ctx.enter_context(tc.tile_pool(name="ost", bufs=2))
        p_mx = ctx.enter_context(tc.tile_pool(name="mx", bufs=q_super))
        p_stat = ctx.enter_context(tc.tile_pool(name="stat", bufs=3 * q_super))
        p_misc = ctx.enter_context(tc.tile_pool(name="misc", bufs=1))
        ps_s = ctx.enter_context(tc.tile_pool(name="psS", bufs=2, space="PSUM"))
        ps_t = ctx.enter_context(tc.tile_pool(name="psT", bufs=2, space="PSUM"))
        ps_o = ctx.enter_context(tc.tile_pool(name="psO", bufs=2, space="PSUM"))

        ident = p_misc.tile([128, 128], F32)
        make_identity(nc, ident[:])

        def nat_slice(kc):  # natural-layout c rows [128k, d] in DRAM
            if kc < KM:
                return m_dram[kc * 128:(kc + 1) * 128, :]
            kk = kc - KM
            return h_dram[kk * 128:(kk + 1) * 128, :]

        cmem_tiles = {}

        # ---- build: transpose c into ct_g[...], one 512-wide group at a
        # time. Groups are emitted lazily, interleaved with the QK loop, so
        # the PE fills DMA latency with either transposes or matmuls and the
        # shared ctkt pool slots alternate between stage and ct tiles.
        pending = set(range(KB // 4))

        def emit_build_group(g, stage_d=None):
            pending.discard(g)
            if stage_d is None:
                stage_d = [p_ctkt.tile([128, 512], F32R, tag="ctd",
                                       name=f"stage{g}_{dc}") for dc in range(DC)]
            for j in range(4):
                kc = g * 4 + j
                cn = p_cn.tile([128, d], F32, tag="cn", name=f"bcn{kc}")
                nc.sync.dma_start(cn[:], nat_slice(kc))
                if kc < cmem_resident:
                    cm = p_cmem.tile([128, d], F32R, tag="cmem",
                                     name=f"cmem{kc}")
                    nc.vector.tensor_copy(cm[:], cn[:])
                    cmem_tiles[kc] = cm
                for dc in range(DC):
                    tp = ps_t.tile([128, 128], F32, tag="psT", name=f"btp{kc}_{dc}")
                    nc.tensor.transpose(tp[:], cn[:, dc * 128:(dc + 1) * 128], ident[:])
                    nc.vector.tensor_copy(stage_d[dc][:, j * 128:(j + 1) * 128], tp[:])
            for dc in range(DC):
                nc.sync.dma_start(ct_g[g][dc, :, :], stage_d[dc][:])
            return stage_d

        def ensure_group(g):
            if g in pending:
                emit_build_group(g)

        g_h0 = (mlen // 512)
        build_queue = [g_h0] + [g for g in range(KB // 4) if g != g_h0]

        def pump_build(min_g=0):
            for g in build_queue:
                if g in pending and g >= min_g:
                    emit_build_group(g)
                    return

        # ---- main loop over q-superblocks
        n_super = (QB + q_super - 1) // q_super
        for s in range(n_super):
            qbs = [s * q_super + j for j in range(q_super) if s * q_super + j < QB]
            nq = len(qbs)
            kr_super = klen_r(qbs[-1])
            nkt_super = kr_super // 512

            # stationary hT for this superblock: cT columns for these queries
            q0 = mlen + qbs[0] * 128
            assert q0 % 512 == 0 and nq * 128 <= 512
            g_ht = q0 // 512
            fused = (s == 0 and nq * 128 == 512)
            ht_d = [p_ht.tile([128, 128 * nq], F32R, tag="ht", name=f"ht{s}_{dc}")
                    for dc in range(DC)]
            if fused:
                # build the query group straight into the hT tiles (layout is
                # identical); emit_build_group also persists it to DRAM
                emit_build_group(g_ht, stage_d=ht_d)
            else:
                ensure_group(g_ht)
                for dc in range(DC):
                    nc.sync.dma_start(
                        ht_d[dc][:],
                        ct_g[g_ht][dc, :, 0:128 * nq],
                    )

            srows = {}
            mxs = {}
            for j, i in enumerate(qbs):
                srows[i] = p_srow.tile([128, kr_super], F32, tag="srow", name=f"srow{i}")
                mxs[i] = p_mx.tile([128, NKT], F32, tag="mx", name=f"mx{i}")

            # QK phase (per-qb softmax stats fire as soon as that q-block's
            # last k-tile has drained, so exp overlaps the remaining QK work)
            stats = {}

            def emit_stats(i):
                nkt_i = klen_r(i) // 512
                negmax = p_stat.tile([128, 1], F32, tag="stat", name=f"negmax{i}")
                nc.vector.tensor_reduce(
                    negmax[:], mxs[i][:, 0:nkt_i],
                    axis=mybir.AxisListType.X, op=mybir.AluOpType.max, negate=True,
                )
                sumv = p_stat.tile([128, 1], F32, tag="stat", name=f"sumv{i}")
                nc.scalar.activation(
                    srows[i][:, 0:klen_r(i)], srows[i][:, 0:klen_r(i)],
                    mybir.ActivationFunctionType.Exp,
                    bias=negmax[:], scale=1.0, accum_out=sumv[:],
                )
                rsum = p_stat.tile([128, 1], F32, tag="stat", name=f"rsum{i}")
                nc.vector.reciprocal(rsum[:], sumv[:])
                stats[i] = rsum

            for kt in range(nkt_super):
                if fused:
                    # ktile kt covers exactly build group kt
                    if kt == g_ht:
                        ct_d = ht_d
                    else:
                        ct_d = emit_build_group(kt)
                    pump_build(min_g=nkt_super)
                else:
                    ensure_group(min(kt + 2, nkt_super - 1))
                    pump_build()
                    ct_d = [p_ctkt.tile([128, 512], F32R, tag="ctd",
                                        name=f"ct{s}_{kt}_{dc}") for dc in range(DC)]
                    for dc in range(DC):
                        nc.sync.dma_start(ct_d[dc][:], ct_g[kt][dc, :, :])
                for j, i in enumerate(qbs):
                    if (kt + 1) * 512 > klen_r(i):
                        continue
                    sps = ps_s.tile([128, 512], F32, tag="psS")
                    for dc in range(DC):
                        nc.tensor.matmul(
                            sps[:],
                            ht_d[dc][:, j * 128:(j + 1) * 128],
                            ct_d[dc][:],
                            start=(dc == 0),
                            stop=(dc == DC - 1),
                        )
                    nc.vector.tensor_reduce(
                        mxs[i][:, kt:kt + 1], sps[:],
                        axis=mybir.AxisListType.X, op=mybir.AluOpType.max,
                    )
                    nc.scalar.copy(srows[i][:, kt * 512:(kt + 1) * 512], sps[:])
                    if (kt + 1) * 512 == klen_r(i):
                        # causal boundary: keep S[r, c] iff c <= r + off
                        off = 128 * i + mlen + 512 - klen_r(i)
                        nc.gpsimd.affine_select(
                            out=srows[i][:, kt * 512:(kt + 1) * 512],
                            in_=srows[i][:, kt * 512:(kt + 1) * 512],
                            compare_op=mybir.AluOpType.is_ge,
                            fill=NEG_INF,
                            base=off,
                            pattern=[[-1, 512]],
                            channel_multiplier=1,
                        )
                        emit_stats(i)

            # PV phase in pairs of q-blocks
            for p0 in range(0, nq, 2):
                pair = qbs[p0:p0 + 2]
                nkc = klen_valid(pair[-1]) // 128
                ops = {i: ps_o.tile([128, d], F32, tag="psO", name=f"opsum{i}") for i in pair}
                for kc in range(nkc):
                    if kc < cmem_resident:
                        cn = cmem_tiles[kc]
                    else:
                        cn = p_cn.tile([128, d], F32R, tag="cn")
                        nc.sync.dma_start(cn[:], nat_slice(kc).bitcast(F32R))
                    for i in pair:
                        last = klen_valid(i) // 128 - 1
                        if kc > last:
                            continue
                        tp = ps_t.tile([128, 128], F32, tag="psT")
                        nc.tensor.transpose(
                            tp[:], srows[i][:, kc * 128:(kc + 1) * 128], ident[:]
                        )
                        pt = p_pt.tile([128, 128], F32R, tag="pt")
                        nc.vector.tensor_copy(pt[:], tp[:])
                        for half in range(d // 512):
                            nc.tensor.matmul(
                                ops[i][:, half * 512:(half + 1) * 512],
                                pt[:],
                                cn[:, half * 512:(half + 1) * 512],
                                start=(kc == 0),
                                stop=(kc == last),
                            )
                for i in pair:
                    ost = p_ost.tile([128, d], F32, tag="ost")
                    nc.vector.tensor_scalar_mul(ost[:], ops[i][:], stats[i][:])
                    nc.sync.dma_start(o_dram[i * 128:(i + 1) * 128, :], ost[:])

    split_waits(nc)
    return nc


_NC_CACHE = {}


def _get_nc(key):
    if key not in _NC_CACHE:
        _NC_CACHE[key] = build_attention(*key)
    return _NC_CACHE[key]


def kernel(h: np.ndarray, mems: np.ndarray) -> np.ndarray:
    qlen, bsz, d = h.shape
    mlen = mems.shape[0]
    nc = _get_nc((qlen, mlen, d))
    in_maps = [
        {
            "h": np.ascontiguousarray(h[:, b, :]),
            "mems": np.ascontiguousarray(mems[:, b, :]),
        }
        for b in range(bsz)
    ]
    res = run_bass_kernel_spmd(nc, in_maps, list(range(bsz))).results
    return np.stack([res[b]["out"] for b in range(bsz)], axis=1)


if __name__ == "__main__":
    rng = np.random.default_rng(0)
    h = rng.standard_normal((QLEN, BSZ, D), dtype=np.float32)
    mems = rng.standard_normal((MLEN, BSZ, D), dtype=np.float32)
    out = kernel(h, mems)
    print("out", out.shape, out.dtype)


# revision 20
# speedup vs baseline: 1.0377x; 1.0377x over previous
"""Trainium2 Bass kernel for nn_Attention_63660005261999.

Reference (per batch element b):
    c = concat(mems[:, b, :], h[:, b, :])           # [klen, d]
    S = h_b @ c_b.T                                  # [qlen, klen]
    S[q, k] = -1e6  where k > q + mlen               # causal w/ memory
    P = softmax(S, axis=-1)
    out_b = P @ c_b                                  # [qlen, d]

Sharding: bsz=8 across 8 NeuronCores, one batch element per core.

Per-core design (two-phase flash attention, fp32 I/O, float32r matmuls):
  build: stream c (mems+h) natural tiles once, PE-transpose into cT
         (d-on-partition) stored in internal DRAM; retain first
         CMEM_RESIDENT k-tiles of natural-layout c in SBUF (f32r view).
  per q-superblock of 4 q-blocks (128 q each):
    QK:  S[qb, :klen_r] accumulated in PSUM over 8 d-chunks,
         lhsT = cT slice (queries), rhs = cT k-tile (keys); running
         per-tile max on DVE, S tiles copied to SBUF by ACT.
         Causal boundary handled by a gpsimd affine_select on the last
         512-wide k-tile; k-tiles beyond the boundary are skipped.
    exp: ACT activation Exp with bias = -rowmax, accum_out = rowsum.
    PV:  per pair of q-blocks: for each 128-wide k-chunk, PE-transpose
         P block, round to f32r in the PSUM->SBUF copy, matmul against
         natural-layout c tile; accumulate O in PSUM; final DVE
         tensor_scalar multiply by 1/rowsum on the way out.

The walrus build in this container accepts at most ONE sync-wait per
instruction; split_waits() rewrites the scheduled module so extra waits
ride on dedicated same-engine NoOps.
"""

import numpy as np
from contextlib import ExitStack

import concourse.bass as bass
import concourse.mybir as mybir
import concourse.tile as tile
from concourse.bass_utils import run_bass_kernel_spmd
from concourse.masks import make_identity

F32 = mybir.dt.float32
F32R = mybir.dt.float32r
NEG_INF = -1000000.0

QLEN, MLEN, BSZ, D = 2048, 2048, 8, 1024
N_CORES = 8
CMEM_RESIDENT = 12  # natural-layout c k-tiles kept resident in SBUF


def split_waits(nc, max_waits: int = 1) -> int:
    """walrus here allows at most one sync wait per instruction; move extras
    onto preceding same-engine NoOp carriers."""
    n_split = 0
    for f in nc.m.functions:
        for blk in f.blocks:
            new_instrs = []
            for ins in blk.instructions:
                si = getattr(ins, "sync_info", None)
                if si is not None and si.on_wait and len(si.on_wait) > max_waits:
                    waits = list(si.on_wait)
                    keep = waits[-max_waits:]
                    spill = waits[:-max_waits]
                    for j, w in enumerate(spill):
                        nop = mybir.InstNoOp(
                            name=f"{ins.name}_wf{j}",
                            text_hint="waitfix",
                            bass_nofuse=True,
                        )
                        nop.engine = ins.engine
                        nop.sync_info = mybir.SyncInfo(on_wait=[w], on_update=[])
                        nc.register_instruction(nop, overwrite=True)
                        new_instrs.append(nop)
                    ins.sync_info = mybir.SyncInfo(
                        on_wait=keep, on_update=list(si.on_update)
                    )
                    n_split += 1
                new_instrs.append(ins)
            blk.instructions[:] = new_instrs
    return n_split


def build_attention(qlen=QLEN, mlen=MLEN, d=D, cmem_resident=CMEM_RESIDENT,
                    q_super=4):
    """One-core attention program: inputs h [qlen, d], mems [mlen, d]."""
    klen = qlen + mlen
    DC = d // 128            # d-chunks
    QB = qlen // 128         # q-blocks
    KB = klen // 128         # k-chunks (natural layout)
    KM = mlen // 128         # k-chunks in mems
    NKT = klen // 512        # 512-wide k-tiles (max)
    assert qlen % 512 == 0 and mlen % 512 == 0 and d % 128 == 0

    def klen_valid(i):       # number of unmasked keys for q-block i
        return mlen + 128 * (i + 1)

    def klen_r(i):           # rounded up to 512-wide k-tiles
        return ((klen_valid(i) + 511) // 512) * 512

    nc = bass.Bass()
    h_dram = nc.declare_dram_parameter("h", [qlen, d], F32, isOutput=False)
    m_dram = nc.declare_dram_parameter("mems", [mlen, d], F32, isOutput=False)
    o_dram = nc.declare_dram_parameter("out", [qlen, d], F32, isOutput=True)
    # one scratch tensor per 512-wide key group so Tile's DRAM dependency
    # tracking (whole-tensor granularity) doesn't serialize QK behind the
    # entire build phase
    ct_g = [nc.dram_tensor(f"ct_g{g}", [DC, 128, 512], F32R)
            for g in range(klen // 512)]

    with tile.TileContext(nc) as tc, ExitStack() as ctx:
        p_cn = ctx.enter_context(tc.tile_pool(name="cn", bufs=3))
        p_cmem = ctx.enter_context(tc.tile_pool(name="cmem", bufs=max(cmem_resident, 1)))
        p_ctkt = ctx.enter_context(tc.tile_pool(name="ctkt", bufs=2 * (d // 128)))
        p_ht = ctx.enter_context(tc.tile_pool(name="ht", bufs=d // 128))
        p_srow = ctx.enter_context(tc.tile_pool(name="srow", bufs=q_super))
        p_pt = ctx.enter_context(tc.tile_pool(name="pt", bufs=4))
        p_ost = ctx.enter_context(tc.tile_pool(name="ost", bufs=2))
        p_mx = ctx.enter_context(tc.tile_pool(name="mx", bufs=q_super))
        p_stat = ctx.enter_context(tc.tile_pool(name="stat", bufs=3 * q_super))
        p_misc = ctx.enter_context(tc.tile_pool(name="misc", bufs=1))
        ps_s = ctx.enter_context(tc.tile_pool(name="psS", bufs=2, space="PSUM"))
        ps_t = ctx.enter_context(tc.tile_pool(name="psT", bufs=2, space="PSUM"))
        ps_o = ctx.enter_context(tc.tile_pool(name="psO", bufs=2, space="PSUM"))

        ident = p_misc.tile([128, 128], F32)
        make_identity(nc, ident[:])

        def nat_slice(kc):  # natural-layout c rows [128k, d] in DRAM
            if kc < KM:
                return m_dram[kc * 128:(kc + 1) * 128, :]
            kk = kc - KM
            return h_dram[kk * 128:(kk + 1) * 128, :]

        cmem_tiles = {}

        # ---- build: transpose c into ct_g[...], one 512-wide group at a
        # time. Groups are emitted lazily, interleaved with the QK loop, so
        # the PE fills DMA latency with either transposes or matmuls and the
        # shared ctkt pool slots alternate between stage and ct tiles.
        pending = set(range(KB // 4))

        def emit_build_group(g, stage_d=None):
            pending.discard(g)
            if stage_d is None:
                stage_d = [p_ctkt.tile([128, 512], F32R, tag="ctd",
                                       name=f"stage{g}_{dc}") for dc in range(DC)]
            for j in range(4):
                kc = g * 4 + j
                cn = p_cn.tile([128, d], F32, tag="cn", name=f"bcn{kc}")
                nc.sync.dma_start(cn[:], nat_slice(kc))
                if kc < cmem_resident:
                    cm = p_cmem.tile([128, d], F32R, tag="cmem",
                                     name=f"cmem{kc}")
                    nc.vector.tensor_copy(cm[:], cn[:])
                    cmem_tiles[kc] = cm
                for dc in range(DC):
                    tp = ps_t.tile([128, 128], F32, tag="psT", name=f"btp{kc}_{dc}")
                    nc.tensor.transpose(tp[:], cn[:, dc * 128:(dc + 1) * 128], ident[:])
                    nc.vector.tensor_copy(stage_d[dc][:, j * 128:(j + 1) * 128], tp[:])
            for dc in range(DC):
                nc.sync.dma_start(ct_g[g][dc, :, :], stage_d[dc][:])
            return stage_d

        def ensure_group(g):
            if g in pending:
                emit_build_group(g)

        g_h0 = (mlen // 512)
        build_queue = [g_h0] + [g for g in range(KB // 4) if g != g_h0]

        def pump_build(min_g=0):
            for g in build_queue:
                if g in pending and g >= min_g:
                    emit_build_group(g)
                    return

        # ---- main loop over q-superblocks
        n_super = (QB + q_super - 1) // q_super
        for s in range(n_super):
            qbs = [s * q_super + j for j in range(q_super) if s * q_super + j < QB]
            nq = len(qbs)
            kr_super = klen_r(qbs[-1])
            nkt_super = kr_super // 512

            # stationary hT for this superblock: cT columns for these queries
            q0 = mlen + qbs[0] * 128
            assert q0 % 512 == 0 and nq * 128 <= 512
            g_ht = q0 // 512
            fused = (s == 0 and nq * 128 == 512)
            ht_d = [p_ht.tile([128, 128 * nq], F32R, tag="ht", name=f"ht{s}_{dc}")
                    for dc in range(DC)]
            if fused:
                # build the query group straight into the hT tiles (layout is
                # identical); emit_build_group also persists it to DRAM
                emit_build_group(g_ht, stage_d=ht_d)
            else:
                ensure_group(g_ht)
                for dc in range(DC):
                    nc.sync.dma_start(
                        ht_d[dc][:],
                        ct_g[g_ht][dc, :, 0:128 * nq],
                    )

            srows = {}
            mxs = {}
            for j, i in enumerate(qbs):
                srows[i] = p_srow.tile([128, kr_super], F32, tag="srow", name=f"srow{i}")
                mxs[i] = p_mx.tile([128, NKT], F32, tag="mx", name=f"mx{i}")

            # QK phase (per-qb softmax stats fire as soon as that q-block's
            # last k-tile has drained, so exp overlaps the remaining QK work)
            stats = {}

            def emit_stats(i):
                nkt_i = klen_r(i) // 512
                negmax = p_stat.tile([128, 1], F32, tag="stat", name=f"negmax{i}")
                nc.vector.tensor_reduce(
                    negmax[:], mxs[i][:, 0:nkt_i],
                    axis=mybir.AxisListType.X, op=mybir.AluOpType.max, negate=True,
                )
                sumv = p_stat.tile([128, 1], F32, tag="stat", name=f"sumv{i}")
                nc.scalar.activation(
                    srows[i][:, 0:klen_r(i)], srows[i][:, 0:klen_r(i)],
                    mybir.ActivationFunctionType.Exp,
                    bias=negmax[:], scale=1.0, accum_out=sumv[:],
                )
                rsum = p_stat.tile([128, 1], F32, tag="stat", name=f"rsum{i}")
                nc.vector.reciprocal(rsum[:], sumv[:])
                stats[i] = rsum

            for kt in range(nkt_super):
                if fused:
                    # ktile kt covers exactly build group kt
                    if kt == g_ht:
                        ct_d = ht_d
                    else:
                        ct_d = emit_build_group(kt)
                    pump_build(min_g=nkt_super)
                else:
                    ensure_group(min(kt + 2, nkt_super - 1))
                    pump_build()
                    ct_d = [p_ctkt.tile([128, 512], F32R, tag="ctd",
                                        name=f"ct{s}_{kt}_{dc}") for dc in range(DC)]
                    for dc in range(DC):
                        nc.sync.dma_start(ct_d[dc][:], ct_g[kt][dc, :, :])
                for j, i in enumerate(qbs):
                    if (kt + 1) * 512 > klen_r(i):
                        continue
                    sps = ps_s.tile([128, 512], F32, tag="psS")
                    for dc in range(DC):
                        nc.tensor.matmul(
                            sps[:],
                            ht_d[dc][:, j * 128:(j + 1) * 128],
                            ct_d[dc][:],
                            start=(dc == 0),
                            stop=(dc == DC - 1),
                        )
                    nc.vector.tensor_reduce(
                        mxs[i][:, kt:kt + 1], sps[:],
                        axis=mybir.AxisListType.X, op=mybir.AluOpType.max,
                    )
                    nc.vector.tensor_copy(srows[i][:, kt * 512:(kt + 1) * 512], sps[:])
                    if (kt + 1) * 512 == klen_r(i):
                        # causal boundary: keep S[r, c] iff c <= r + off
                        off = 128 * i + mlen + 512 - klen_r(i)
                        nc.gpsimd.affine_select(
                            out=srows[i][:, kt * 512:(kt + 1) * 512],
                            in_=srows[i][:, kt * 512:(kt + 1) * 512],
                            compare_op=mybir.AluOpType.is_ge,
                            fill=NEG_INF,
                            base=off,
                            pattern=[[-1, 512]],
                            channel_multiplier=1,
                        )
                        emit_stats(i)

            # PV phase in pairs of q-blocks
            for p0 in range(0, nq, 2):
                pair = qbs[p0:p0 + 2]
                nkc = klen_valid(pair[-1]) // 128
                ops = {i: ps_o.tile([128, d], F32, tag="psO", name=f"opsum{i}") for i in pair}
                for kc in range(nkc):
                    if kc < cmem_resident:
                        cn = cmem_tiles[kc]
                    else:
                        cn = p_cn.tile([128, d], F32R, tag="cn")
                        nc.sync.dma_start(cn[:], nat_slice(kc).bitcast(F32R))
                    for i in pair:
                        last = klen_valid(i) // 128 - 1
                        if kc > last:
                            continue
                        tp = ps_t.tile([128, 128], F32, tag="psT")
                        nc.tensor.transpose(
                            tp[:], srows[i][:, kc * 128:(kc + 1) * 128], ident[:]
                        )
                        pt = p_pt.tile([128, 128], F32R, tag="pt")
                        nc.vector.tensor_copy(pt[:], tp[:])
                        for half in range(d // 512):
                            nc.tensor.matmul(
                                ops[i][:, half * 512:(half + 1) * 512],
                                pt[:],
                                cn[:, half * 512:(half + 1) * 512],
                                start=(kc == 0),
                                stop=(kc == last),
                            )
                for i in pair:
                    ost = p_ost.tile([128, d], F32, tag="ost")
                    nc.vector.tensor_scalar_mul(ost[:], ops[i][:], stats[i][:])
                    nc.sync.dma_start(o_dram[i * 128:(i + 1) * 128, :], ost[:])

    split_waits(nc)
    return nc


_NC_CACHE = {}


def _get_nc(key):
    if key not in _NC_CACHE:
        _NC_CACHE[key] = build_attention(*key)
    return _NC_CACHE[key]


def kernel(h: np.ndarray, mems: np.ndarray) -> np.ndarray:
    qlen, bsz, d = h.shape
    mlen = mems.shape[0]
    nc = _get_nc((qlen, mlen, d))
    in_maps = [
        {
            "h": np.ascontiguousarray(h[:, b, :]),
            "mems": np.ascontiguousarray(mems[:, b, :]),
        }
        for b in range(bsz)
    ]
    res = run_bass_kernel_spmd(nc, in_maps, list(range(bsz))).results
    return np.stack([res[b]["out"] for b in range(bsz)], axis=1)


if __name__ == "__main__":
    rng = np.random.default_rng(0)
    h = rng.standard_normal((QLEN, BSZ, D), dtype=np.float32)
    mems = rng.standard_normal((MLEN, BSZ, D), dtype=np.float32)
    out = kernel(h, mems)
    print("out", out.shape, out.dtype)
